# revision 1
# baseline (speedup 1.0000x reference)
"""Single-head causal attention (B=8, T=4096, EMB=1024, HEAD=64) on 8 trn2 cores.

Strategy: data-parallel over batch, one batch element per NeuronCore.

Per core (all matmuls in bf16, fp32 PSUM accumulation):
  1. QKV projection from host-pretransposed xT [1024, 4096]:
       KQ^T [128, 4096]  (rows 0:64 = K^T, 64:128 = Q^T), via W[:, 0:128] stationary
       V    [4096, 64]   natural layout, via xT-chunk stationary x Wv moving
  2. Q^T copied to partitions 0:63 (sbuf->sbuf DMA) so scores matmuls can
     contract over d=64 on partitions 0:63.
  3. Scores S^T[s, t] = K @ Q^T per (s-chunk 128, t-tile 512), PSUM fp32.
     No max-subtraction: scores ~ N(0, 0.41), exp is safe in fp32.
  4. exp via ScalarE directly from PSUM (scale=1/8 folded in), bf16 out.
     Causal: only tiles with t >= s computed; diagonal 128x128 block masked
     by a 0/1 multiply after exp.
  5. PV: P^T tile [128s, 128t] stationary, V-with-ones-column [128, 65] moving
     -> accumulates [O_unnorm | rowsum] in natural [t, 65] layout.
  6. out = O_unnorm * reciprocal(rowsum) (per-partition scalar), DMA out fp32.
"""

from contextlib import ExitStack

import numpy as np
import ml_dtypes

B, T, EMB, HEAD = 8, 4096, 1024, 64
KCH = EMB // 128          # 8 contraction chunks
NTT = T // 512            # 8 t-tiles of 512
NTS = T // 128            # 32 t-subtiles / s-chunks of 128
BF16 = ml_dtypes.bfloat16

_CACHE = {}


def _build_program():
    import concourse.bacc as bacc
    import concourse.tile as tile
    from concourse import mybir
    from concourse.masks import make_identity

    fp32 = mybir.dt.float32
    bf16 = mybir.dt.bfloat16
    EXP = mybir.ActivationFunctionType.Exp

    nc = bacc.Bacc("TRN2", target_bir_lowering=False, debug=False)
    xt_ap = nc.dram_tensor("xt", [EMB, T], bf16, kind="ExternalInput").ap()
    w_ap = nc.dram_tensor("w", [EMB, 192], bf16, kind="ExternalInput").ap()
    mask_ap = nc.dram_tensor("mask", [128, 128], bf16, kind="ExternalInput").ap()
    o_ap = nc.dram_tensor("o", [T, HEAD], fp32, kind="ExternalOutput").ap()

    with tile.TileContext(nc) as tc:
        with (
            tc.tile_pool(name="consts", bufs=1) as consts,
            tc.tile_pool(name="outs", bufs=4) as outs,
        ):
            # ---------- constants ----------
            w_sb = consts.tile([128, KCH, 192], bf16, tag="w")
            for k in range(KCH):
                nc.sync.dma_start(out=w_sb[:, k, :], in_=w_ap[k * 128:(k + 1) * 128, :])
            mask_sb = consts.tile([128, 128], bf16, tag="mask")
            nc.sync.dma_start(out=mask_sb, in_=mask_ap)
            ident_sb = consts.tile([128, 128], fp32, tag="ident")
            make_identity(nc, ident_sb)
            # V with ones column: [128, 65] per s-chunk; col 64 preset to 1.0
            vt_sb = consts.tile([128, NTS * 65], bf16, tag="vt")
            nc.gpsimd.memset(vt_sb, 1.0)

            kq_sb = consts.tile([128, T], bf16, tag="kq")
            qk_sb = consts.tile([128, T], bf16, tag="qk")

            # ---------- phase 1: load x, project (k-outer: PE starts as
            # each x chunk lands; KQ in two half-T passes + V share 8 banks)
            with (
                tc.tile_pool(name="xp", bufs=1) as xp,
                tc.tile_pool(name="ps_kq", bufs=1, space="PSUM") as ps_kq,
                tc.tile_pool(name="ps_v", bufs=1, space="PSUM") as ps_v,
            ):
                xt_sb = xp.tile([128, KCH, T], bf16, tag="xt")
                for k in range(KCH):
                    nc.sync.dma_start(
                        out=xt_sb[:, k, :], in_=xt_ap[k * 128:(k + 1) * 128, :]
                    )

                pkq = []
                for j in range(4):
                    pkq_j = ps_kq.tile([128, 512], fp32, tag=f"kq{j}")
                    pkq.append(pkq_j)
                pv = []
                for g in range(4):
                    pv_g = ps_v.tile([128, 512], fp32, tag=f"v{g}")
                    pv.append(pv_g)

                # pass 1 over k: KQ t-tiles 0..3 and all V accumulators
                for k in range(KCH):
                    for j in range(4):
                        nc.tensor.matmul(
                            pkq[j],
                            w_sb[:, k, 0:128],
                            xt_sb[:, k, j * 512:(j + 1) * 512],
                            start=(k == 0),
                            stop=(k == KCH - 1),
                            skip_group_check=True,
                        )
                    for i in range(NTS):
                        # start=True clears the WHOLE bank's has_written bits,
                        # so only the first accumulator in each bank may issue
                        # it; the rest overwrite-on-first-write via per-element
                        # has_written.
                        nc.tensor.matmul(
                            pv[i // 8][:, (i % 8) * 64:(i % 8 + 1) * 64],
                            xt_sb[:, k, i * 128:(i + 1) * 128],
                            w_sb[:, k, 128:192],
                            start=(k == 0 and i % 8 == 0),
                            stop=(k == KCH - 1),
                            skip_group_check=True,
                        )
                for j in range(4):
                    nc.vector.tensor_copy(kq_sb[:, j * 512:(j + 1) * 512], pkq[j])
                for i in range(NTS):
                    nc.vector.tensor_copy(
                        vt_sb[:, i * 65:i * 65 + 64],
                        pv[i // 8][:, (i % 8) * 64:(i % 8 + 1) * 64],
                    )
                # pass 2 over k (x fully resident): KQ t-tiles 4..7
                pkq2 = []
                for j in range(4):
                    pkq2_j = ps_kq.tile([128, 512], fp32, tag=f"kq{j}")
                    pkq2.append(pkq2_j)
                for k in range(KCH):
                    for j in range(4):
                        nc.tensor.matmul(
                            pkq2[j],
                            w_sb[:, k, 0:128],
                            xt_sb[:, k, (j + 4) * 512:(j + 5) * 512],
                            start=(k == 0),
                            stop=(k == KCH - 1),
                            skip_group_check=True,
                        )
                for j in range(4):
                    nc.vector.tensor_copy(
                        kq_sb[:, (j + 4) * 512:(j + 5) * 512], pkq2[j]
                    )
                # Q^T to low partitions for scores moving operand
                nc.sync.dma_start(out=qk_sb[0:64, :], in_=kq_sb[64:128, :])

            # ---------- phase 2: attention ----------
            phase2 = ExitStack()
            ptp = phase2.enter_context(tc.tile_pool(name="pt", bufs=1))
            ps_s = phase2.enter_context(tc.tile_pool(name="ps_s", bufs=2, space="PSUM"))
            ps_o = phase2.enter_context(tc.tile_pool(name="ps_o", bufs=1, space="PSUM"))
            pt = []
            for a in range(NTS):
                pt_a = ptp.tile([128, T - 128 * a], bf16, tag=f"pt{a}")
                pt.append(pt_a)

            def score_groups(a):
                """[(jstart, gsize), ...] groups of <=3 t-tiles for s-chunk a."""
                j0 = a // 4
                groups = []
                j = j0
                while j < NTT:
                    g = min(3, NTT - j)
                    groups.append((j, g))
                    j += g
                return groups

            def emit_scores(a):
                tiles = []
                for (jstart, g) in score_groups(a):
                    psg = ps_s.tile([128, 512 * g], fp32, tag="sg")
                    for idx in range(g):
                        j = jstart + idx
                        nc.tensor.matmul(
                            psg[:, idx * 512:(idx + 1) * 512],
                            kq_sb[0:64, a * 128:(a + 1) * 128],
                            qk_sb[0:64, j * 512:(j + 1) * 512],
                            start=True,
                            stop=True,
                        )
                    tiles.append((jstart, g, psg))
                return tiles

            def emit_exp(a, tiles):
                for (jstart, g, psg) in tiles:
                    skip = max(0, 128 * a - 512 * jstart)
                    out_lo = 512 * jstart + skip - 128 * a
                    out_hi = 512 * (jstart + g) - 128 * a
                    nc.scalar.activation(
                        pt[a][:, out_lo:out_hi],
                        psg[:, skip:512 * g],
                        EXP,
                        scale=0.125,
                    )
                # mask the diagonal 128x128 block (zero where s > t)
                nc.vector.tensor_mul(pt[a][:, 0:128], pt[a][:, 0:128], mask_sb)

            def emit_pv(i):
                po = ps_o.tile([128, 65], fp32, tag="o")
                for aa in range(i + 1):
                    nc.tensor.matmul(
                        po,
                        pt[aa][:, 128 * (i - aa):128 * (i - aa) + 128],
                        vt_sb[:, aa * 65:(aa + 1) * 65],
                        start=(aa == 0),
                        stop=(aa == i),
                    )
                dr = outs.tile([128, 1], fp32, tag="dr")
                nc.vector.reciprocal(dr, po[:, 64:65])
                o_sb = outs.tile([128, 64], fp32, tag="o_sb")
                nc.vector.tensor_scalar_mul(o_sb, po[:, 0:64], dr)
                nc.sync.dma_start(out=o_ap[i * 128:(i + 1) * 128, :], in_=o_sb)

            # software-pipelined: while ACT(a) drains, PE runs S(a+1); PV for
            # t-tile j fires once its last needed chunk (4j+3) is exp'd.
            tiles = emit_scores(0)
            for a in range(NTS):
                emit_exp(a, tiles)
                if a + 1 < NTS:
                    tiles = emit_scores(a + 1)
                if a >= 1:
                    emit_pv(a - 1)
            emit_pv(NTS - 1)
            phase2.close()

    nc.compile()
    return nc


def _get_nc():
    if "nc" not in _CACHE:
        _CACHE["nc"] = _build_program()
    return _CACHE["nc"]


def kernel(x, W):
    from concourse.bass_utils import run_bass_kernel_spmd

    x = np.asarray(x, dtype=np.float32)
    W = np.asarray(W, dtype=np.float32)
    assert x.shape == (B, T, EMB) and W.shape == (EMB, 3 * HEAD)

    xt = np.ascontiguousarray(x.transpose(0, 2, 1)).astype(BF16)  # [B, EMB, T]
    w16 = W.astype(BF16)
    mask = np.triu(np.ones((128, 128), np.float32)).astype(BF16)

    nc = _get_nc()
    in_maps = [{"xt": xt[b], "w": w16, "mask": mask} for b in range(B)]
    res = run_bass_kernel_spmd(nc, in_maps, list(range(B)))
    return np.stack([res.results[b]["o"] for b in range(B)]).astype(np.float32)



# revision 4
# speedup vs baseline: 1.1648x; 1.1648x over previous
"""Single-head causal attention (B=8, T=4096, EMB=1024, HEAD=64) on 8 trn2 cores.

Strategy: data-parallel over batch, one batch element per NeuronCore.

Per core (all matmuls in bf16, fp32 PSUM accumulation):
  1. x loaded as pre-transposed xT [1024, 4096] bf16, chunks split across the
     two HWDGE DMA queues (sync + scalar) to cut the load-phase from ~28us.
  2. KQ^T [128, 4096] (rows 0:64 = K^T, 64:128 = Q^T) in ONE pass over k
     (8 PSUM banks, k-outer j-inner).  At the last k step, each bank is
     copied out right after its final matmul, and SBUF->SBUF DMA copies
     build the swapped tile qk2 = [Q^T; K^T] in 1024-col slices so scores
     can start early.
  3. Scores use PE row-tiling (contraction d=64 only): even t-tiles run in
     array rows 0:63 (K^T lo x Q^T lo from kq/qk2), odd t-tiles in rows
     64:127 (K^T hi x Q^T hi) - two matmuls in flight -> ~2x scores rate.
  4. exp via ScalarE from PSUM (scale=1/8 folded), bf16 out, groups of <=2
     PSUM banks per instruction.  Causal: only tiles with t >= s computed;
     diagonal 128x128 block masked by a 0/1 multiply after exp.
  5. V projection (xT chunk stationary x Wv moving -> natural [t, 64])
     emitted in 2-row-block quarters interleaved into the attention loop so
     it fills PE slack instead of delaying the first exp.
  6. PV: P^T tile [128s, 128t] stationary, V-with-ones-column [128, 65]
     moving -> accumulates [O_unnorm | rowsum] in natural [t, 65] layout.
  7. out = O_unnorm * reciprocal(rowsum) (per-partition scalar), DMA out.
"""

from contextlib import ExitStack

import numpy as np
import ml_dtypes

B, T, EMB, HEAD = 8, 4096, 1024, 64
KCH = EMB // 128          # 8 contraction chunks
NTT = T // 512            # 8 t-tiles of 512
NTS = T // 128            # 32 t-subtiles / s-chunks of 128
BF16 = ml_dtypes.bfloat16

_CACHE = {}


def _build_program():
    import concourse.bacc as bacc
    import concourse.tile as tile
    from concourse import mybir

    fp32 = mybir.dt.float32
    bf16 = mybir.dt.bfloat16
    EXP = mybir.ActivationFunctionType.Exp

    nc = bacc.Bacc("TRN2", target_bir_lowering=False, debug=False)
    xt_ap = nc.dram_tensor("xt", [EMB, T], bf16, kind="ExternalInput").ap()
    w_ap = nc.dram_tensor("w", [EMB, 192], bf16, kind="ExternalInput").ap()
    mask_ap = nc.dram_tensor("mask", [128, 128], bf16, kind="ExternalInput").ap()
    o_ap = nc.dram_tensor("o", [T, HEAD], fp32, kind="ExternalOutput").ap()

    with tile.TileContext(nc) as tc:
        with (
            tc.tile_pool(name="consts", bufs=1) as consts,
            tc.tile_pool(name="outs", bufs=4) as outs,
        ):
            # ---------- constants ----------
            mask_sb = consts.tile([128, 128], bf16, tag="mask")
            nc.sync.dma_start(out=mask_sb, in_=mask_ap)
            w_sb = consts.tile([128, KCH, 192], bf16, tag="w")
            for k in range(KCH):
                nc.sync.dma_start(out=w_sb[:, k, :], in_=w_ap[k * 128:(k + 1) * 128, :])
            # warm the ACT exp table while DMAs run (first ACTIVATE of a set
            # pays ~2.7us of table-load otherwise right before the first real
            # exp on the critical path)
            warm = outs.tile([128, 1], fp32, tag="warm")
            nc.scalar.activation(warm, mask_sb[:, 0:1], EXP)
            # V with ones column: [128, 65] per s-chunk; col 64 preset to 1.0
            vt_sb = consts.tile([128, NTS * 65], bf16, tag="vt")
            nc.gpsimd.memset(vt_sb, 1.0)

            kq_sb = consts.tile([128, T], bf16, tag="kq")    # [K^T; Q^T]
            qk2_sb = consts.tile([128, T], bf16, tag="qk2")  # [Q^T; K^T]

            # ---------- phase 1: load x, project KQ ----------
            xstack = ExitStack()
            xp = xstack.enter_context(
                tc.tile_pool(name="xp", bufs=1, side="right")
            )
            xt_sb = xp.tile([128, KCH, T], bf16, tag="xt")
            for k in range(KCH):
                eng = nc.sync if k % 2 == 0 else nc.scalar
                eng.dma_start(
                    out=xt_sb[:, k, :], in_=xt_ap[k * 128:(k + 1) * 128, :]
                )

            with tc.tile_pool(name="ps_kq", bufs=1, space="PSUM") as ps_kq:
                pkq = []
                for j in range(NTT):
                    pkq_j = ps_kq.tile([128, 512], fp32, tag=f"kq{j}")
                    pkq.append(pkq_j)
                for k in range(KCH):
                    for j in range(NTT):
                        nc.tensor.matmul(
                            pkq[j],
                            w_sb[:, k, 0:128],
                            xt_sb[:, k, j * 512:(j + 1) * 512],
                            start=(k == 0),
                            stop=(k == KCH - 1),
                            skip_group_check=True,
                        )
                        if k == KCH - 1:
                            # bank j is final: drain it while the remaining
                            # last-k matmuls run
                            nc.vector.tensor_copy(
                                kq_sb[:, j * 512:(j + 1) * 512], pkq[j]
                            )
                            if j % 2 == 1:
                                # swapped-layout slices for row-tiled scores
                                lo = (j - 1) * 512
                                hi = (j + 1) * 512
                                nc.gpsimd.dma_start(
                                    out=qk2_sb[0:64, lo:hi],
                                    in_=kq_sb[64:128, lo:hi],
                                )
                                nc.scalar.dma_start(
                                    out=qk2_sb[64:128, lo:hi],
                                    in_=kq_sb[0:64, lo:hi],
                                )

            # ---------- phase 2: attention (+ interleaved V projection) ----
            phase2 = ExitStack()
            ptA = phase2.enter_context(tc.tile_pool(name="ptA", bufs=1))
            ps_s = phase2.enter_context(tc.tile_pool(name="ps_s", bufs=2, space="PSUM"))
            ps_v = phase2.enter_context(tc.tile_pool(name="ps_v", bufs=2, space="PSUM"))
            ps_o = phase2.enter_context(tc.tile_pool(name="ps_o", bufs=1, space="PSUM"))
            pt = [None] * NTS
            ptB_pool = [None]

            vstate = {"blk": None}

            def emit_vquarter(q):
                """V projection for rows i = 2q, 2q+1 (k-inner, 8-bank-free)."""
                if q % 4 == 0:
                    vstate["blk"] = ps_v.tile(
                        [128, 512], fp32, tag="vblk", name=f"vblk{q // 4}"
                    )
                blk = vstate["blk"]
                for i in (2 * q, 2 * q + 1):
                    c0 = (i % 8) * 64
                    for k in range(KCH):
                        nc.tensor.matmul(
                            blk[:, c0:c0 + 64],
                            xt_sb[:, k, i * 128:(i + 1) * 128],
                            w_sb[:, k, 128:192],
                            start=(k == 0 and i % 8 == 0),
                            stop=(k == KCH - 1),
                            skip_group_check=True,
                        )
                for i in (2 * q, 2 * q + 1):
                    c0 = (i % 8) * 64
                    nc.vector.tensor_copy(
                        vt_sb[:, i * 65:i * 65 + 64], blk[:, c0:c0 + 64]
                    )

            def score_groups(a):
                """[(jstart, gsize), ...] groups of <=2 t-tiles for s-chunk a."""
                j0 = a // 4
                groups = []
                j = j0
                while j < NTT:
                    g = min(2, NTT - j)
                    groups.append((j, g))
                    j += g
                return groups

            def emit_scores(a):
                tiles = []
                for (jstart, g) in score_groups(a):
                    psg = ps_s.tile([128, 512 * g], fp32, tag="sg",
                                    padded_shape=[128, 1024],
                                    name=f"sg{a}_{jstart}")
                    for idx in range(g):
                        j = jstart + idx
                        if j % 2 == 0:
                            # PE rows 0:63 - K^T lo stationary, Q^T lo moving
                            nc.tensor.matmul(
                                psg[:, idx * 512:(idx + 1) * 512],
                                kq_sb[0:64, a * 128:(a + 1) * 128],
                                qk2_sb[0:64, j * 512:(j + 1) * 512],
                                start=True,
                                stop=True,
                            )
                        else:
                            # PE rows 64:127 - K^T hi stationary, Q^T hi moving
                            nc.tensor.matmul(
                                psg[:, idx * 512:(idx + 1) * 512],
                                qk2_sb[64:128, a * 128:(a + 1) * 128],
                                kq_sb[64:128, j * 512:(j + 1) * 512],
                                start=True,
                                stop=True,
                            )
                    tiles.append((jstart, g, psg))
                return tiles

            def emit_exp(a, tiles):
                pool = ptA if a < 16 else ptB_pool[0]
                pt[a] = pool.tile([128, T - 128 * a], bf16, tag=f"pt{a}",
                                  name=f"pt{a}")
                for (jstart, g, psg) in tiles:
                    skip = max(0, 128 * a - 512 * jstart)
                    out_lo = 512 * jstart + skip - 128 * a
                    out_hi = 512 * (jstart + g) - 128 * a
                    nc.scalar.activation(
                        pt[a][:, out_lo:out_hi],
                        psg[:, skip:512 * g],
                        EXP,
                        scale=0.125,
                    )
                # mask the diagonal 128x128 block (zero where s > t)
                nc.vector.tensor_mul(pt[a][:, 0:128], pt[a][:, 0:128], mask_sb)

            def emit_pv(i):
                po = ps_o.tile([128, 65], fp32, tag="o")
                for aa in range(i + 1):
                    nc.tensor.matmul(
                        po,
                        pt[aa][:, 128 * (i - aa):128 * (i - aa) + 128],
                        vt_sb[:, aa * 65:(aa + 1) * 65],
                        start=(aa == 0),
                        stop=(aa == i),
                    )
                dr = outs.tile([128, 1], fp32, tag="dr")
                nc.vector.reciprocal(dr, po[:, 64:65])
                o_sb = outs.tile([128, 64], fp32, tag="o_sb")
                nc.vector.tensor_scalar_mul(o_sb, po[:, 0:64], dr)
                nc.sync.dma_start(out=o_ap[i * 128:(i + 1) * 128, :], in_=o_sb)

            # fill the PE gap between the last KQ matmul and the first
            # scores matmul (which waits on the qk2 copies) with V work
            for q in (0, 1, 2):
                emit_vquarter(q)

            # software-pipelined: while ACT(a) drains, PE runs S(a+1); PV for
            # t-tile i fires once chunk i is exp'd; V quarters fill PE slack.
            tiles = emit_scores(0)
            for a in range(NTS):
                if a == 16:
                    # all V work done (last xt use was quarter 15 at a=12):
                    # free the x tile and place the remaining P tiles there
                    xstack.close()
                    ptB_pool[0] = phase2.enter_context(
                        tc.tile_pool(name="ptB", bufs=1)
                    )
                emit_exp(a, tiles)
                if a + 1 < NTS:
                    tiles = emit_scores(a + 1)
                if a >= 1:
                    emit_pv(a - 1)
                if a + 3 <= 15:
                    emit_vquarter(a + 3)
            emit_pv(NTS - 1)
            phase2.close()

    nc.compile()
    return nc


def _get_nc():
    if "nc" not in _CACHE:
        _CACHE["nc"] = _build_program()
    return _CACHE["nc"]


def kernel(x, W):
    from concourse.bass_utils import run_bass_kernel_spmd

    x = np.asarray(x, dtype=np.float32)
    W = np.asarray(W, dtype=np.float32)
    assert x.shape == (B, T, EMB) and W.shape == (EMB, 3 * HEAD)

    xt = np.ascontiguousarray(x.transpose(0, 2, 1)).astype(BF16)  # [B, EMB, T]
    w16 = W.astype(BF16)
    mask = np.triu(np.ones((128, 128), np.float32)).astype(BF16)

    nc = _get_nc()
    in_maps = [{"xt": xt[b], "w": w16, "mask": mask} for b in range(B)]
    res = run_bass_kernel_spmd(nc, in_maps, list(range(B)))
    return np.stack([res.results[b]["o"] for b in range(B)]).astype(np.float32)


# revision 9
# speedup vs baseline: 1.1806x; 1.0136x over previous
"""Single-head causal attention (B=8, T=4096, EMB=1024, HEAD=64) on 8 trn2 cores.

Strategy: data-parallel over batch, one batch element per NeuronCore.

Per core (all matmuls in bf16, fp32 PSUM accumulation):
  1. x loaded as pre-transposed xT [1024, 4096] bf16, chunks split across the
     two HWDGE DMA queues (sync + scalar) to cut the load-phase from ~28us.
  2. KQ^T [128, 4096] (rows 0:64 = K^T, 64:128 = Q^T) in ONE pass over k
     (8 PSUM banks, k-outer j-inner).  At the last k step, each bank is
     copied out right after its final matmul, and SBUF->SBUF DMA copies
     build the swapped tile qk2 = [Q^T; K^T] in 1024-col slices so scores
     can start early.
  3. Scores use PE row-tiling (contraction d=64 only): even t-tiles run in
     array rows 0:63 (K^T lo x Q^T lo from kq/qk2), odd t-tiles in rows
     64:127 (K^T hi x Q^T hi) - two matmuls in flight -> ~2x scores rate.
  4. exp via ScalarE from PSUM (scale=1/8 folded), bf16 out, groups of <=2
     PSUM banks per instruction.  Causal: only tiles with t >= s computed;
     diagonal 128x128 block masked by a 0/1 multiply after exp.
  5. V projection (xT chunk stationary x Wv moving -> natural [t, 64])
     emitted in 2-row-block quarters interleaved into the attention loop so
     it fills PE slack instead of delaying the first exp.
  6. PV: P^T tile [128s, 128t] stationary, V-with-ones-column [128, 65]
     moving -> accumulates [O_unnorm | rowsum] in natural [t, 65] layout.
  7. out = O_unnorm * reciprocal(rowsum) (per-partition scalar), DMA out.
"""

from contextlib import ExitStack

import numpy as np
import ml_dtypes

B, T, EMB, HEAD = 8, 4096, 1024, 64
KCH = EMB // 128          # 8 contraction chunks
NTT = T // 512            # 8 t-tiles of 512
NTS = T // 128            # 32 t-subtiles / s-chunks of 128
BF16 = ml_dtypes.bfloat16

_CACHE = {}


def _build_program():
    import concourse.bacc as bacc
    import concourse.tile as tile
    from concourse import mybir

    fp32 = mybir.dt.float32
    bf16 = mybir.dt.bfloat16
    EXP = mybir.ActivationFunctionType.Exp

    nc = bacc.Bacc("TRN2", target_bir_lowering=False, debug=False)
    xt_ap = nc.dram_tensor("xt", [EMB, T], bf16, kind="ExternalInput").ap()
    w_ap = nc.dram_tensor("w", [EMB, 192], bf16, kind="ExternalInput").ap()
    mask_ap = nc.dram_tensor("mask", [128, 128], bf16, kind="ExternalInput").ap()
    o_ap = nc.dram_tensor("o", [T, HEAD], fp32, kind="ExternalOutput").ap()

    with tile.TileContext(nc) as tc:
        with (
            tc.tile_pool(name="consts", bufs=1) as consts,
            tc.tile_pool(name="outs", bufs=4) as outs,
        ):
            # ---------- phase 1: x chunks FIRST, alone on the sync HWDGE
            # queue.  One [128, 4096] instruction per chunk keeps all 16 SDMA
            # engines on one in-order ring: chunk k lands every ~2.5us
            # (~350 GB/s) and the KQ k-loop consumes chunks as they arrive.
            # Emitting anything on sync before these would also steal the 8
            # DMA-completion semaphore lanes and serialize the chunk loads.
            xstack = ExitStack()
            xp = xstack.enter_context(
                tc.tile_pool(name="xp", bufs=1, side="right")
            )
            xt_sb = xp.tile([128, KCH, T], bf16, tag="xt")
            for k in range(KCH):
                nc.sync.dma_start(
                    out=xt_sb[:, k, :], in_=xt_ap[k * 128:(k + 1) * 128, :]
                )

            # ---------- constants (scalar queue - idle until phase 2) ------
            mask_sb = consts.tile([128, 128], bf16, tag="mask")
            nc.scalar.dma_start(out=mask_sb, in_=mask_ap)
            w_sb = consts.tile([128, KCH, 192], bf16, tag="w")
            for k in range(KCH):
                nc.scalar.dma_start(
                    out=w_sb[:, k, :], in_=w_ap[k * 128:(k + 1) * 128, :]
                )
            # warm the ACT exp table while DMAs run (first ACTIVATE of a set
            # pays ~2.7us of table-load otherwise right before the first real
            # exp on the critical path)
            warm = outs.tile([128, 1], fp32, tag="warm")
            nc.scalar.activation(warm, mask_sb[:, 0:1], EXP)
            # V with ones column: [128, s-chunk, 65]; col 64 preset to 1.0
            vt_sb = consts.tile([128, NTS, 65], bf16, tag="vt")
            nc.gpsimd.memset(vt_sb, 1.0)

            kq_sb = consts.tile([128, T], bf16, tag="kq")    # [K^T; Q^T]
            qk2_sb = consts.tile([128, T], bf16, tag="qk2")  # [Q^T; K^T]

            with tc.tile_pool(name="ps_kq", bufs=1, space="PSUM") as ps_kq:
                pkq = []
                for j in range(NTT):
                    pkq_j = ps_kq.tile([128, 512], fp32, tag=f"kq{j}")
                    pkq.append(pkq_j)
                for k in range(KCH):
                    for j in range(NTT):
                        nc.tensor.matmul(
                            pkq[j],
                            w_sb[:, k, 0:128],
                            xt_sb[:, k, j * 512:(j + 1) * 512],
                            start=(k == 0),
                            stop=(k == KCH - 1),
                            skip_group_check=True,
                        )
                        if k == KCH - 1:
                            # bank j is final: drain it while the remaining
                            # last-k matmuls run
                            nc.vector.tensor_copy(
                                kq_sb[:, j * 512:(j + 1) * 512], pkq[j]
                            )
                            if j % 2 == 1:
                                # swapped-layout slices for row-tiled scores
                                lo = (j - 1) * 512
                                hi = (j + 1) * 512
                                nc.gpsimd.dma_start(
                                    out=qk2_sb[0:64, lo:hi],
                                    in_=kq_sb[64:128, lo:hi],
                                )
                                nc.scalar.dma_start(
                                    out=qk2_sb[64:128, lo:hi],
                                    in_=kq_sb[0:64, lo:hi],
                                )
            del pkq

            # ---------- phase 2: attention (+ interleaved V projection) ----
            phase2 = ExitStack()
            ptA = phase2.enter_context(tc.tile_pool(name="ptA", bufs=1))
            ps_s = phase2.enter_context(tc.tile_pool(name="ps_s", bufs=2, space="PSUM"))
            ps_v = phase2.enter_context(tc.tile_pool(name="ps_v", bufs=2, space="PSUM"))
            ps_o = phase2.enter_context(tc.tile_pool(name="ps_o", bufs=2, space="PSUM"))
            pt = [None] * NTS
            ptB_pool = [None]

            vstate = {"blk": None}

            def emit_vquarter(q):
                """V projection for rows i = 2q, 2q+1 (k-inner, 1 bank/block)."""
                if q % 4 == 0:
                    vstate["blk"] = ps_v.tile(
                        [128, 8, 64], fp32, tag="vblk", name=f"vblk{q // 4}"
                    )
                blk = vstate["blk"]
                for i in (2 * q, 2 * q + 1):
                    for k in range(KCH):
                        nc.tensor.matmul(
                            blk[:, i % 8, :],
                            xt_sb[:, k, i * 128:(i + 1) * 128],
                            w_sb[:, k, 128:192],
                            start=(k == 0 and i % 8 == 0),
                            stop=(k == KCH - 1),
                            skip_group_check=True,
                        )
                if q % 4 == 3:
                    # one strided copy per 8-row block: fewer PE<->DVE PSUM
                    # bank serializations than per-row copies
                    b = q // 4
                    nc.vector.tensor_copy(
                        vt_sb[:, 8 * b:8 * b + 8, 0:64], blk
                    )

            def score_groups(a):
                """[(jstart, gsize), ...] groups of <=2 t-tiles for s-chunk a."""
                j0 = a // 4
                groups = []
                j = j0
                while j < NTT:
                    g = min(2, NTT - j)
                    groups.append((j, g))
                    j += g
                return groups

            def emit_scores(a):
                tiles = []
                for (jstart, g) in score_groups(a):
                    psg = ps_s.tile([128, 512 * g], fp32, tag="sg",
                                    padded_shape=[128, 1024],
                                    name=f"sg{a}_{jstart}")
                    for idx in range(g):
                        j = jstart + idx
                        if j % 2 == 0:
                            # PE rows 0:63 - K^T lo stationary, Q^T lo moving
                            nc.tensor.matmul(
                                psg[:, idx * 512:(idx + 1) * 512],
                                kq_sb[0:64, a * 128:(a + 1) * 128],
                                qk2_sb[0:64, j * 512:(j + 1) * 512],
                                start=True,
                                stop=True,
                            )
                        else:
                            # PE rows 64:127 - K^T hi stationary, Q^T hi moving
                            nc.tensor.matmul(
                                psg[:, idx * 512:(idx + 1) * 512],
                                qk2_sb[64:128, a * 128:(a + 1) * 128],
                                kq_sb[64:128, j * 512:(j + 1) * 512],
                                start=True,
                                stop=True,
                            )
                    tiles.append((jstart, g, psg))
                return tiles

            def emit_exp(a, tiles):
                pool = ptA if a < 16 else ptB_pool[0]
                pt[a] = pool.tile([128, T - 128 * a], bf16, tag=f"pt{a}",
                                  name=f"pt{a}")
                for (jstart, g, psg) in tiles:
                    skip = max(0, 128 * a - 512 * jstart)
                    out_lo = 512 * jstart + skip - 128 * a
                    out_hi = 512 * (jstart + g) - 128 * a
                    nc.scalar.activation(
                        pt[a][:, out_lo:out_hi],
                        psg[:, skip:512 * g],
                        EXP,
                        scale=0.125,
                    )
                # mask the diagonal 128x128 block (zero where s > t)
                nc.vector.tensor_mul(pt[a][:, 0:128], pt[a][:, 0:128], mask_sb)

            def emit_pv(i):
                po = ps_o.tile([128, 65], fp32, tag="o", name=f"po{i}",
                               padded_shape=[128, 512])
                for aa in range(i + 1):
                    nc.tensor.matmul(
                        po,
                        pt[aa][:, 128 * (i - aa):128 * (i - aa) + 128],
                        vt_sb[:, aa, :],
                        start=(aa == 0),
                        stop=(aa == i),
                    )
                dr = outs.tile([128, 1], fp32, tag="dr")
                nc.vector.reciprocal(dr, po[:, 64:65])
                o_sb = outs.tile([128, 64], fp32, tag="o_sb")
                nc.vector.tensor_scalar_mul(o_sb, po[:, 0:64], dr)
                nc.sync.dma_start(out=o_ap[i * 128:(i + 1) * 128, :], in_=o_sb)

            # fill the PE gap between the last KQ matmul and the first
            # scores matmul (which waits on the qk2 copies) with V work
            for q in (0, 1, 2):
                emit_vquarter(q)

            # software-pipelined: while ACT(a) drains, PE runs S(a+1); PV for
            # t-tile i fires once chunk i is exp'd; V quarters fill PE slack.
            tiles = emit_scores(0)
            for a in range(NTS):
                if a == 16:
                    # all V work done (last xt use was quarter 15 at a=12):
                    # free the x tile and place the remaining P tiles there
                    xstack.close()
                    ptB_pool[0] = phase2.enter_context(
                        tc.tile_pool(name="ptB", bufs=1)
                    )
                emit_exp(a, tiles)
                if a + 1 < NTS:
                    tiles = emit_scores(a + 1)
                if a >= 1:
                    emit_pv(a - 1)
                if a + 3 <= 15:
                    emit_vquarter(a + 3)
            emit_pv(NTS - 1)
            phase2.close()

    nc.compile()
    return nc


def _get_nc():
    if "nc" not in _CACHE:
        _CACHE["nc"] = _build_program()
    return _CACHE["nc"]


def kernel(x, W):
    from concourse.bass_utils import run_bass_kernel_spmd

    x = np.asarray(x, dtype=np.float32)
    W = np.asarray(W, dtype=np.float32)
    assert x.shape == (B, T, EMB) and W.shape == (EMB, 3 * HEAD)

    xt = np.ascontiguousarray(x.transpose(0, 2, 1)).astype(BF16)  # [B, EMB, T]
    w16 = W.astype(BF16)
    mask = np.triu(np.ones((128, 128), np.float32)).astype(BF16)

    nc = _get_nc()
    in_maps = [{"xt": xt[b], "w": w16, "mask": mask} for b in range(B)]
    res = run_bass_kernel_spmd(nc, in_maps, list(range(B)))
    return np.stack([res.results[b]["o"] for b in range(B)]).astype(np.float32)
